# revision 37
# baseline (speedup 1.0000x reference)
"""Trainium2 Bass kernel for nn_MultiHeadAttention_75737453297867.

Sharding: one head per NeuronCore (8 heads / 8 cores). The reference's
aliased as_strided gather needs a per-core base offset 24576*h into the
flange-padded k/v storage; 24576*h mod 9216 is always row-aligned
(phi in {0,48,96} rows of the padded [144,64] channel), so each core
statically builds three phi-shifted staging variants and the active one
is selected purely through host data: the QK contraction runs over a
stacked K=12 (q-conv weights zeroed for inactive variants) and uv goes
through a one-hot select matmul. No runtime-dependent addressing.

Per core: q conv (12 stacked ch), k/v convs (4 ch slice c_lo..c_lo+4,
k pre-scaled by DPH^-0.5), 3-variant staging in DRAM, static gathers,
block attention (QK on PE fp32r, exp on ACT from PSUM, AV on PE with a
ones-row denominator), final conv with wo[:, 4h:4h+4]. Host sums the 8
partial outputs.
"""

import sys

import numpy as np

if "/opt/trn_rl_repo" not in sys.path:
    sys.path.insert(0, "/opt/trn_rl_repo")

import concourse.bass as bass
import concourse.tile as tile
from concourse import bacc
from concourse import mybir
from concourse.bass_types import AP

# Problem constants
CIN, COUT, H, W = 64, 64, 128, 48
DM, NH, DPH = 32, 8, 4
Q0, Q1, F0, F1 = 128, 24, 8, 8
M0, M1 = Q0 + 2 * F0, Q1 + 2 * F1          # 144, 40
HP, WP = H + 2 * F0, W + 2 * F1            # 144, 64
KV = M0 * M1                               # 5760
NKC = 48                                   # kv chunks of 120 (3 m0-rows)
KC = 120
NQC = 8                                    # q chunks of 384 (16 rows x 24 cols)
QC = 384
QROWS = 16
RPB = 3                                    # kv chunks per psum round (3 banks, double-buffered)
PHIS = (0, 48, 96)
F32 = mybir.dt.float32
F32R = mybir.dt.float32r
I32 = mybir.dt.int32


def build_nc(debug=False):
    nc = bacc.Bacc()
    dbg = {}
    if debug:
        dbg['q'] = nc.dram_tensor("dbg_q", [12, H * W], F32, kind="ExternalOutput")
        dbg['k'] = nc.dram_tensor("dbg_k", [4, H * W], F32, kind="ExternalOutput")
        dbg['uk0'] = nc.dram_tensor("dbg_uk0", [12, KV], F32, kind="ExternalOutput")
        dbg['uvt0'] = nc.dram_tensor("dbg_uvt0", [KC, NKC * 5], F32, kind="ExternalOutput")
        dbg['opad'] = nc.dram_tensor("dbg_opad", [4, 130 * 50], F32, kind="ExternalOutput")

    xp_d = nc.dram_tensor("xp", [CIN, 130 * 50], F32R, kind="ExternalInput")
    wq_d = nc.dram_tensor("wq_t", [CIN, 9 * 12], F32R, kind="ExternalInput")
    wk_d = nc.dram_tensor("wk_t", [CIN, 9 * 4], F32R, kind="ExternalInput")
    wv_d = nc.dram_tensor("wv_t", [CIN, 9 * 4], F32R, kind="ExternalInput")
    wo_d = nc.dram_tensor("wo_t", [5, 9 * 64], F32R, kind="ExternalInput")
    bq_d = nc.dram_tensor("bq_l", [12, 1], F32, kind="ExternalInput")
    bk_d = nc.dram_tensor("bk_l", [4, 1], F32, kind="ExternalInput")
    bv_d = nc.dram_tensor("bv_l", [4, 1], F32, kind="ExternalInput")
    sel_d = nc.dram_tensor("sel", [12, 4], F32R, kind="ExternalInput")
    id_d = nc.dram_tensor("ident4", [4, 4], F32, kind="ExternalInput")
    cc_d = nc.dram_tensor("concol", [128, 2], F32, kind="ExternalInput")
    o14_d = nc.dram_tensor("ones14", [1, 5], F32R, kind="ExternalInput")
    out_d = nc.dram_tensor("out", [COUT, H * W], F32, kind="ExternalOutput")

    from contextlib import ExitStack

    with tile.TileContext(nc) as tc, ExitStack() as ctx:
        P = ctx.enter_context(tc.tile_pool(name="persist", bufs=1))
        dram = ctx.enter_context(tc.tile_pool(name="dram", bufs=1, space="DRAM"))
        ctx1 = ctx.enter_context(ExitStack())
        P1 = ctx1.enter_context(tc.tile_pool(name="phase1", bufs=1))

        # ---- load constants ----
        xp_sb = P1.tile([CIN, 130, 50], F32R, tag="xp")
        nc.sync.dma_start(out=xp_sb, in_=xp_d[:, :].rearrange("p (a b) -> p a b", a=130))
        wq_sb = P.tile([CIN, 9, 12], F32R, tag="wq")
        nc.sync.dma_start(out=wq_sb, in_=wq_d[:, :].rearrange("p (t o) -> p t o", t=9))
        wk_sb = P.tile([CIN, 9, 4], F32R, tag="wk")
        nc.sync.dma_start(out=wk_sb, in_=wk_d[:, :].rearrange("p (t o) -> p t o", t=9))
        wv_sb = P.tile([CIN, 9, 4], F32R, tag="wv")
        nc.sync.dma_start(out=wv_sb, in_=wv_d[:, :].rearrange("p (t o) -> p t o", t=9))
        wo_sb = P.tile([5, 9, 64], F32R, tag="wo")
        nc.sync.dma_start(out=wo_sb, in_=wo_d[:, :].rearrange("p (t o) -> p t o", t=9))
        bq_sb = P.tile([12, 1], F32, tag="bq")
        nc.sync.dma_start(out=bq_sb, in_=bq_d[:, :])
        bk_sb = P.tile([4, 1], F32, tag="bk")
        nc.sync.dma_start(out=bk_sb, in_=bk_d[:, :])
        bv_sb = P.tile([4, 1], F32, tag="bv")
        nc.sync.dma_start(out=bv_sb, in_=bv_d[:, :])
        sel_sb = P.tile([12, 4], F32R, tag="sel")
        nc.sync.dma_start(out=sel_sb, in_=sel_d[:, :])
        ident = P.tile([4, 4], F32, tag="ident")
        nc.sync.dma_start(out=ident, in_=id_d[:, :])
        concol = P.tile([128, 2], F32, tag="concol")
        nc.sync.dma_start(out=concol, in_=cc_d[:, :])
        ones14 = P.tile([1, 5], F32R, tag="ones14")
        nc.sync.dma_start(out=ones14, in_=o14_d[:, :])

        # ---- convs: q (12ch stacked), k (4ch, pre-scaled), v (4ch) ----
        q_sb = P.tile([12, 128, 48], F32R, tag="q_sb")
        k_sb = P1.tile([4, 128, 48], F32, tag="k_sb")
        v_sb = P1.tile([4, 128, 48], F32, tag="v_sb")

        with tc.tile_pool(name="psc", bufs=4, space="PSUM") as psc:
            for w_sb, b_sb, m, dst in (
                (wq_sb, bq_sb, 12, q_sb),
                (wk_sb, bk_sb, 4, k_sb),
                (wv_sb, bv_sb, 4, v_sb),
            ):
                for chv in range(16):          # 16 chunks of 8 rows
                    ps = psc.tile([m, 8, 48], F32, tag="cps")
                    for t in range(9):
                        dy, dx = t // 3, t % 3
                        rhs = xp_sb[:, 8 * chv + dy : 8 * chv + dy + 8, dx : dx + 48]
                        nc.tensor.matmul(
                            ps[:, :, :], w_sb[:, t, 0:m], rhs,
                            start=(t == 0), stop=(t == 8),
                        )
                    nc.vector.tensor_scalar_add(
                        dst[:, 8 * chv : 8 * chv + 8, :], ps[:, :, :],
                        b_sb[0:m, 0:1],
                    )

        # ---- 3-variant phi-shifted staging in DRAM ----
        zero_sb = P1.tile([3, 2304], F32, tag="zeros")
        nc.vector.memset(zero_sb, 0.0)
        kp_drs, vp_drs = [], []
        for v, phi in enumerate(PHIS):
            for src_sb, lst, nm in ((k_sb, kp_drs, "kp"), (v_sb, vp_drs, "vp")):
                buf = dram.tile([3, 144 * 64], F32, tag=f"{nm}{v}")
                for t in range(4):
                    nc.sync.dma_start(
                        out=buf[:, 2304 * t : 2304 * (t + 1)], in_=zero_sb
                    )
                bv_ = buf[:, :].rearrange("p (a b) -> p a b", a=144)
                if phi == 0:
                    nc.gpsimd.dma_start(
                        out=bv_[:, 8:136, 8:56], in_=src_sb[0:3, :, :]
                    )
                else:
                    nc.gpsimd.dma_start(
                        out=bv_[:, 0 : 136 - phi, 8:56],
                        in_=src_sb[0:3, phi - 8 : 128, :],
                    )
                    nc.gpsimd.dma_start(
                        out=bv_[:, 152 - phi : 144, 8:56],
                        in_=src_sb[1:4, 0 : phi - 8, :],
                    )
                lst.append(buf)

        # ---- static gathers: uk_stack/uv_stack [12, 144, 40] per window ----
        uk_rep = []
        uvT = []
        for j in range(2):
            ukr = P.tile([12, M0, M1], F32R, tag=f"ukrep{j}")
            uvstk = P1.tile([12, M0, M1], F32R, tag="uvstack")
            for v in range(3):
                for buf, dst in ((kp_drs[v], ukr), (vp_drs[v], uvstk)):
                    src = AP(
                        tensor=buf.tensor,
                        offset=buf.offset + 24 * j,
                        ap=[[6144, 4], [48, M0], [1, M1]],
                    )
                    nc.gpsimd.dma_start(
                        out=dst[4 * v : 4 * v + 4, :, :], in_=src.bitcast(F32R)
                    )
            uk_rep.append(ukr)

            # one-hot select of the active variant's uv: [4, 5760]
            uvs = P1.tile([4, M0, M1], F32, tag="uvs")
            uvs_f = uvs.rearrange("p a b -> p (a b)")
            uvstk_f = uvstk[:, :, :].rearrange("p a b -> p (a b)")
            with tc.tile_pool(name=f"pssel{j}", bufs=2, space="PSUM") as pssel:
                for t in range(12):
                    pss = pssel.tile([4, 480], F32, tag="pss")
                    nc.tensor.matmul(
                        pss, sel_sb[:, :], uvstk_f[:, 480 * t : 480 * (t + 1)],
                        start=True, stop=True,
                    )
                    nc.vector.tensor_copy(uvs_f[:, 480 * t : 480 * (t + 1)], pss)

            # transpose uv chunks to [120, 48, 5]; col 4 = ones
            uvt = P.tile([KC, NKC, 5], F32R, tag=f"uvt{j}")
            ones_b = AP(tensor=concol.tensor, offset=concol[0:KC, 1:2].offset,
                        ap=[concol[0:KC, 1:2].ap[0], [0, NKC]])
            nc.vector.tensor_copy(uvt[:, :, 0], ones_b)
            with tc.tile_pool(name=f"pst{j}", bufs=2, space="PSUM") as pst:
                for c in range(NKC):
                    tp = pst.tile([KC, 4], F32, tag="tp")
                    nc.tensor.transpose(
                        tp, uvs_f[:, KC * c : KC * (c + 1)], ident[:, :]
                    )
                    nc.vector.tensor_copy(uvt[:, c, 1:5], tp)
            uvT.append(uvt)

        if debug:
            nc.sync.dma_start(out=dbg['q'][:, :], in_=q_sb[:, :, :].rearrange("p a b -> p (a b)").bitcast(F32))
            nc.sync.dma_start(out=dbg['k'][:, :], in_=k_sb[:, :, :].rearrange("p a b -> p (a b)"))
            nc.sync.dma_start(out=dbg['uk0'][:, :], in_=uk_rep[0][:, :, :].rearrange("p a b -> p (a b)").bitcast(F32))
            nc.sync.dma_start(out=dbg['uvt0'][:, :], in_=uvT[0][:, :, :].rearrange("p a b -> p (a b)").bitcast(F32))

        ctx1.close()  # free phase1 SBUF
        PL = ctx.enter_context(tc.tile_pool(name="late", bufs=1))

        # ---- attention ----
        o_pad = PL.tile([5, 130, 50], F32R, tag="opad")
        zero_b = AP(tensor=concol.tensor, offset=concol[0:5, 0:1].offset,
                    ap=[concol[0:5, 0:1].ap[0], [0, 130 * 50]])
        nc.vector.tensor_copy(o_pad[:, :, :].rearrange("p a b -> p (a b)"), zero_b)

        with (
            tc.tile_pool(name="psqk", bufs=2, space="PSUM") as psqk,
            tc.tile_pool(name="psav", bufs=1, space="PSUM") as psav,
            tc.tile_pool(name="psden", bufs=1, space="PSUM") as psden,
            tc.tile_pool(name="expp", bufs=2) as expp,
            tc.tile_pool(name="redp", bufs=2) as redp,
        ):
            for j in range(2):
                uk_f = uk_rep[j][:, :, :].rearrange("p a b -> p (a b)")
                for qc in range(NQC):
                    ps_av = psav.tile([128, QC], F32, tag="av")
                    for rnd in range(NKC // RPB):       # 8 rounds of 6 chunks
                        ps_qk = psqk.tile([128, RPB, 512], F32, tag="qk")
                        for b in range(RPB):
                            c = RPB * rnd + b           # kv chunk id
                            rhs = q_sb[
                                :,
                                QROWS * qc : QROWS * (qc + 1),
                                24 * j : 24 * j + 24,
                            ]
                            out = ps_qk[0:KC, b, 0:QC].rearrange(
                                "p (a c) -> p a c", a=QROWS
                            )
                            nc.tensor.matmul(
                                out,
                                uk_f[:, KC * c : KC * (c + 1)],
                                rhs,
                                start=True, stop=True,
                            )
                        ex = expp.tile([KC, RPB, QC], F32R, tag="ex")
                        nc.scalar.activation(
                            ex, ps_qk[0:KC, :, 0:QC],
                            mybir.ActivationFunctionType.Exp,
                        )
                        for b in range(RPB):
                            c = RPB * rnd + b
                            nc.tensor.matmul(
                                ps_av[0:5, :],
                                uvT[j][:, c, :],
                                ex[:, b, :],
                                start=(c == 0), stop=(c == NKC - 1),
                            )
                    # normalize and write into o_pad interior
                    s0 = redp.tile([5, QC], F32, tag="s0")
                    nc.vector.tensor_copy(s0, ps_av[0:5, :])
                    rec1 = redp.tile([1, QC], F32, tag="rec1")
                    nc.vector.reciprocal(rec1, s0[0:1, :])
                    rec1r = redp.tile([1, QC], F32R, tag="rec1r")
                    nc.vector.tensor_copy(rec1r, rec1)
                    ps_den = psden.tile([5, QC], F32, tag="den")
                    nc.tensor.matmul(ps_den, ones14[:, :], rec1r,
                                     start=True, stop=True)
                    o_div = redp.tile([5, QC], F32, tag="odiv")
                    nc.vector.tensor_tensor(out=o_div, in0=s0[:, :],
                                            in1=ps_den,
                                            op=mybir.AluOpType.mult)
                    dst = o_pad[
                        :, 1 + QROWS * qc : 1 + QROWS * (qc + 1), 1 + 24 * j : 25 + 24 * j
                    ]
                    nc.vector.tensor_copy(
                        dst, o_div.rearrange("p (a c) -> p a c", a=QROWS)
                    )

        if debug:
            nc.sync.dma_start(out=dbg['opad'][:, :], in_=o_pad[:, :, :].rearrange("p a b -> p (a b)").bitcast(F32))

        # ---- final conv ----
        out_sb = PL.tile([COUT, H, W], F32, tag="outsb")
        with tc.tile_pool(name="psf", bufs=4, space="PSUM") as psf:
            for chv in range(16):
                ps = psf.tile([COUT, 8, 48], F32, tag="fps")
                for t in range(9):
                    dy, dx = t // 3, t % 3
                    rhs = o_pad[:, 8 * chv + dy : 8 * chv + dy + 8, dx : dx + 48]
                    nc.tensor.matmul(
                        ps[:, :, :], wo_sb[:, t, :], rhs,
                        start=(t == 0), stop=(t == 8),
                    )
                nc.vector.tensor_copy(out_sb[:, 8 * chv : 8 * chv + 8, :], ps)
        nc.sync.dma_start(
            out=out_d[:, :], in_=out_sb.rearrange("p a b -> p (a b)")
        )

    nc.compile()
    return nc


_NC = None


def _get_nc():
    global _NC
    if _NC is None:
        _NC = build_nc()
    return _NC


def make_in_maps(x, wq, bq, wk, bk, wv, bv, wo):
    x = np.asarray(x, np.float32)[0]           # [64, 128, 48]
    xp = np.zeros((CIN, 130, 50), np.float32)
    xp[:, 1:129, 1:49] = x
    xp = xp.reshape(CIN, -1)
    s = np.float32(DPH ** -0.5)

    def taps(w):                                # [O, I, 3, 3] -> [I, 9, O]
        return np.ascontiguousarray(np.transpose(w, (1, 2, 3, 0)).reshape(
            w.shape[1], 9, w.shape[0]))

    wq_np = np.asarray(wq, np.float32)
    wk_np = np.asarray(wk, np.float32) * s
    wv_np = np.asarray(wv, np.float32)
    wo_np = np.asarray(wo, np.float32)
    bq_np = np.asarray(bq, np.float32)
    bk_np = np.asarray(bk, np.float32) * s
    bv_np = np.asarray(bv, np.float32)

    in_maps = []
    for h in range(8):
        c_lo = (24576 * h) // 9216
        phi = (24576 * h - 9216 * c_lo) // 64
        v_idx = PHIS.index(phi)

        wq_stack = np.zeros((12, CIN, 3, 3), np.float32)
        wq_stack[4 * v_idx : 4 * v_idx + 4] = wq_np[4 * h : 4 * h + 4]
        bq_stack = np.zeros((12,), np.float32)
        bq_stack[4 * v_idx : 4 * v_idx + 4] = bq_np[4 * h : 4 * h + 4]
        sel = np.zeros((12, 4), np.float32)
        sel[4 * v_idx : 4 * v_idx + 4] = np.eye(4, dtype=np.float32)

        wo_t4 = np.ascontiguousarray(
            np.transpose(wo_np[:, 4 * h : 4 * h + 4], (1, 2, 3, 0))
        ).reshape(4, -1)
        wo_t = np.concatenate([np.zeros((1, wo_t4.shape[1]), np.float32),
                               wo_t4], axis=0)
        in_maps.append({
            "xp": xp,
            "wq_t": taps(wq_stack).reshape(CIN, -1),
            "wk_t": taps(wk_np[c_lo : c_lo + 4]).reshape(CIN, -1),
            "wv_t": taps(wv_np[c_lo : c_lo + 4]).reshape(CIN, -1),
            "wo_t": wo_t,
            "bq_l": bq_stack.reshape(12, 1),
            "bk_l": bk_np[c_lo : c_lo + 4].reshape(4, 1),
            "bv_l": bv_np[c_lo : c_lo + 4].reshape(4, 1),
            "sel": sel,
            "ident4": np.eye(4, dtype=np.float32),
            "concol": np.stack([np.zeros(128, np.float32),
                                np.ones(128, np.float32)], axis=1),
            "ones14": np.ones((1, 5), np.float32),
        })
    return in_maps


def kernel(x, wq, bq, wk, bk, wv, bv, wo):
    from concourse.bass_utils import run_bass_kernel_spmd

    nc = _get_nc()
    in_maps = make_in_maps(x, wq, bq, wk, bk, wv, bv, wo)
    res = run_bass_kernel_spmd(nc, in_maps, list(range(8))).results
    out = np.zeros((COUT, H * W), np.float32)
    for m in res:
        out = out + m["out"]
    return out.reshape(1, COUT, H, W)
